# revision 21
# baseline (speedup 1.0000x reference)
"""PEER / product-key MoE routing kernel for Trainium2 (8 NeuronCores).

Strategy: data-parallel over tokens. Each of the 8 cores gets 256 of the
2048 tokens plus a full replica of the expert tables in its DRAM. Routing
(q projection, product-key scores, two-stage top-8), expert-row gathers,
and the PEER combine all run on-device. No collectives are needed; the
host only slices/packs inputs and concatenates the per-core outputs.

Per-core pipeline (v5):
  PE:  qT = Wq^T @ x^T with 128-wide feature tiles (fp32, exact);
       per-head scores via ONE matmul against block-diagonal keys.
  DVE: top-8 of each 256-score PSUM half via max8/max_index (exact),
       then top-8 of the 8x8 combo sums; winners' sub-key ids resolved
       with an is_equal one-hot reduction.
  PE+DVE: sub-key ids are shuffled into dma_gather's wrapped index
       layout ([16 partitions, n/16] int16) with 8 tiny permutation
       matmuls per token block (values <= 255, bf16-exact), then
       combined to expert ids and biased by -32768 (bitwise xor) so the
       17-bit id range fits int16 (the gather ucode uses a signed MAC;
       the source AP is pre-offset by +32768 rows to compensate).
  GPSIMD: ONE dma_gather per 8-slot group (1024 expert-row-pairs + 1
       pad index, 2 MB) - vectorized descriptor emission replaces 64
       per-slot indirect DMAs whose fixed costs starved the SDMA
       engines (the old path burned 150us of GpSimd time).
  DVE/ACT: inner products as bf16 multiply + free-dim-sum, split
       between the fused DVE scalar_tensor_tensor path and the
       DVE-mult + ACT-accum path to balance the two engines.
  PE:  combine as PSUM-accumulated diag(vals) @ w_up_row matmuls; the
       8 diag matrices of a group are built in one DVE op.

Routing is computed entirely in fp32, so expert selection matches the
fp32 reference exactly; only the expert tables are bf16 (rel err ~4e-3).
"""

import numpy as np

import concourse.bass as bass
import concourse.mybir as mybir
from concourse import bacc
from concourse import library_config
from concourse.bass import IndirectOffsetOnAxis
from concourse.tile import TileContext
from concourse.bass_utils import run_bass_kernel_spmd

N_CORES = 8
N_HEADS = 8
D_KEYS = 128
HALF = 64
N_KEYS = 256
TOP_K = 8
D = 512
B = 2048           # total tokens
BC = B // N_CORES  # tokens per core (256)
TB = BC // 128     # token blocks per core (2)
# dma_gather is limited to 1024 indices per op and trims trailing
# negative (biased) indices. Each op covers one head's first 7 slots
# (n = 897: 896 real + 1 pad index 0 at position 896 defeats the trim;
# mid-list negatives are fine - validity is position-based). The 8th
# slot of each head is gathered with a plain single-offset indirect
# DMA, which is exact and cheap at 128 descriptors.
NOPS = 8           # dma_gather ops per token block (one per head)
NCOL = 57          # wrapped idx columns per op (ceil(897/16))
F32 = mybir.dt.float32
U16 = mybir.dt.uint16
I16 = mybir.dt.int16
I32 = mybir.dt.int32
BF16 = mybir.dt.bfloat16
X = mybir.AxisListType.X
OP = mybir.AluOpType

# inner-product engine split within each 8-slot group:
# slots < STT_SPLIT use the fused DVE scalar_tensor_tensor, the rest use
# DVE-mult + ACT-accum (balances DVE vs Scalar engine load)
STT_SPLIT = 1


def build_nc(use_dg=True):
    nc = bacc.Bacc("TRN2", target_bir_lowering=False)

    xtokb_d = nc.dram_tensor("xtokb", [BC, D], BF16, kind="ExternalInput")
    xt_d = nc.dram_tensor("xt", [D, BC], F32, kind="ExternalInput")
    wq_d = nc.dram_tensor("wq", [D, N_HEADS * D_KEYS], F32, kind="ExternalInput")
    bqp_d = nc.dram_tensor("bqp", [D_KEYS, N_HEADS], F32, kind="ExternalInput")
    kbd_d = nc.dram_tensor("kbd", [D_KEYS, N_HEADS, 2, N_KEYS], F32,
                           kind="ExternalInput")
    wb_d = nc.dram_tensor("wb", [N_KEYS * N_KEYS, 2 * D], BF16,
                          kind="ExternalInput")
    id01_d = nc.dram_tensor("id01", [128, 128], BF16, kind="ExternalInput")
    shf_d = nc.dram_tensor("shf", [128, 8, 128], BF16, kind="ExternalInput")
    iota8_d = nc.dram_tensor("iota8", [128, 8], U16, kind="ExternalInput")
    out_d = nc.dram_tensor("out", [BC, D], F32, kind="ExternalOutput")

    with TileContext(nc) as tc:
        with (
            tc.tile_pool(name="const", bufs=1) as cpool,
            tc.tile_pool(name="psq", bufs=2, space="PSUM") as psq,
            tc.tile_pool(name="pss", bufs=2, space="PSUM") as pss,
            tc.tile_pool(name="pshf", bufs=2, space="PSUM") as pshfp,
            tc.tile_pool(name="st2", bufs=1) as st2,
            tc.tile_pool(name="eqs", bufs=2) as eqs,
            tc.tile_pool(name="wbp", bufs=4 if use_dg else 32) as wbp,
            tc.tile_pool(name="scr", bufs=4) as scrp,
            tc.tile_pool(name="dgp", bufs=4) as dgp,
            tc.tile_pool(name="pacc", bufs=1, space="PSUM") as paccp,
            tc.tile_pool(name="accp", bufs=2) as accp,
        ):
            if use_dg:
                nc.gpsimd.load_library(library_config.mlp)
            # ---- constant loads ----
            wq_sb = []
            xt_sb = []
            for k in range(4):
                t = cpool.tile([128, N_HEADS * D_KEYS], F32, tag=f"wq{k}")
                nc.sync.dma_start(out=t[:], in_=wq_d[k * 128:(k + 1) * 128, :])
                wq_sb.append(t)
                t2 = cpool.tile([128, BC], F32, tag=f"xt{k}")
                nc.sync.dma_start(out=t2[:], in_=xt_d[k * 128:(k + 1) * 128, :])
                xt_sb.append(t2)
            xtok_bf = []
            for tb in range(TB):
                tb16 = cpool.tile([128, D], BF16, tag=f"xtokb{tb}")
                nc.sync.dma_start(out=tb16[:], in_=xtokb_d[tb * 128:(tb + 1) * 128, :])
                xtok_bf.append(tb16)
            kbd_sb = cpool.tile([D_KEYS, N_HEADS, 2, N_KEYS], F32, tag="kbd")
            nc.sync.dma_start(out=kbd_sb[:], in_=kbd_d[:, :, :, :])
            id01_sb = cpool.tile([128, 128], BF16, tag="id01")
            nc.sync.dma_start(out=id01_sb[:], in_=id01_d[:, :])
            shf_sb = cpool.tile([128, 8, 128], BF16, tag="shf")
            nc.sync.dma_start(out=shf_sb[:], in_=shf_d[:, :, :])
            bqp_sb = cpool.tile([D_KEYS, N_HEADS], F32, tag="bqp")
            nc.sync.dma_start(out=bqp_sb[:], in_=bqp_d[:, :])
            iota8 = cpool.tile([128, 8], U16, tag="iota8")
            nc.sync.dma_start(out=iota8[:], in_=iota8_d[:, :])

            qts = [cpool.tile([D_KEYS, N_HEADS, 128], F32, tag=f"qt{tb}",
                              name=f"qt{tb}") for tb in range(TB)]
            widx = {}    # (tb, H) -> wrapped-index tile [128, 4, NCOL] U16
            idx32 = {}   # (tb, op) -> [128, 1] I32 offsets
            ws = {tb: st2.tile([128, 64], F32, tag=f"w8{tb}", name=f"w8{tb}")
                  for tb in range(TB)}
            pages = {}

            def qproj(tb):
                tsl = slice(tb * 128, (tb + 1) * 128)
                qt = qts[tb]
                # ---- qT per head: [feature-in-head, token] (fp32, exact) ----
                for m in range(N_HEADS):
                    ps = psq.tile([128, 128], F32, tag="psq")
                    for k in range(4):
                        nc.tensor.matmul(
                            out=ps[:],
                            lhsT=wq_sb[k][:, m * 128:(m + 1) * 128],
                            rhs=xt_sb[k][:, tsl],
                            start=(k == 0),
                            stop=(k == 3),
                        )
                    nc.vector.tensor_scalar(
                        out=qt[:, m, :], in0=ps[:],
                        scalar1=bqp_sb[:, m:m + 1], scalar2=None, op0=OP.add,
                    )

            stage1 = {}

            def scores_stage(tb, H):
                # scores + stage-1 top8 for heads 4H..4H+3 (exact)
                qt = qts[tb]
                s1t = st2.tile([128, 32], F32, tag=f"s1t{tb}{H}",
                               name=f"s1t{tb}{H}")
                s2t = st2.tile([128, 32], F32, tag=f"s2t{tb}{H}",
                               name=f"s2t{tb}{H}")
                i1 = st2.tile([128, 32], U16, tag=f"i1{tb}{H}",
                              name=f"i1{tb}{H}")
                i2 = st2.tile([128, 32], U16, tag=f"i2{tb}{H}",
                              name=f"i2{tb}{H}")
                for mm in range(4):
                    m = 4 * H + mm
                    ps2 = pss.tile([128, 2, N_KEYS], F32, tag="pss")
                    nc.tensor.matmul(
                        out=ps2[:, :, :].rearrange("p a b -> p (a b)"),
                        lhsT=qt[:, m, :],
                        rhs=kbd_sb[:, m, :, :].rearrange("p a b -> p (a b)"),
                        start=True, stop=True,
                    )
                    for half, (st_, ix) in enumerate(((s1t, i1), (s2t, i2))):
                        nc.vector.max(out=st_[:, mm * 8:(mm + 1) * 8],
                                      in_=ps2[:, half, :])
                        nc.vector.max_index(
                            out=ix[:, mm * 8:(mm + 1) * 8],
                            in_max=st_[:, mm * 8:(mm + 1) * 8],
                            in_values=ps2[:, half, :],
                        )
                stage1[(tb, H)] = (s1t, s2t, i1, i2)

            def resolve_half(tb, H):
                s1t, s2t, i1, i2 = stage1[(tb, H)]
                # ---- stage-2: 8x8 combo scores, top8 of 64 per head ----
                cs = st2.tile([128, 256], F32, tag=f"cs{tb}{H}",
                              name=f"cs{tb}{H}")
                for mm in range(4):
                    nc.vector.tensor_tensor(
                        out=cs[:, mm * 64:(mm + 1) * 64].rearrange(
                            "p (a b) -> p a b", a=8),
                        in0=s1t[:, mm * 8:(mm + 1) * 8].unsqueeze(2)
                            .to_broadcast([128, 8, 8]),
                        in1=s2t[:, mm * 8:(mm + 1) * 8].unsqueeze(1)
                            .to_broadcast([128, 8, 8]),
                        op=OP.add,
                    )
                v8 = st2.tile([128, 32], F32, tag=f"v8{tb}{H}",
                              name=f"v8{tb}{H}")
                n8 = st2.tile([128, 32], U16, tag=f"n8{tb}{H}",
                              name=f"n8{tb}{H}")
                for mm in range(4):
                    nc.vector.max(out=v8[:, mm * 8:(mm + 1) * 8],
                                  in_=cs[:, mm * 64:(mm + 1) * 64])
                    nc.vector.max_index(
                        out=n8[:, mm * 8:(mm + 1) * 8],
                        in_max=v8[:, mm * 8:(mm + 1) * 8],
                        in_values=cs[:, mm * 64:(mm + 1) * 64])
                k1 = st2.tile([128, 32], U16, tag=f"k1{tb}{H}",
                              name=f"k1{tb}{H}")
                nc.vector.tensor_scalar(
                    out=k1[:], in0=n8[:], scalar1=3, scalar2=None,
                    op0=OP.logical_shift_right)
                k2 = st2.tile([128, 32], U16, tag=f"k2{tb}{H}",
                              name=f"k2{tb}{H}")
                nc.vector.tensor_scalar(
                    out=k2[:], in0=n8[:], scalar1=7, scalar2=None,
                    op0=OP.bitwise_and)

                # resolve winners' sub-key ids: isel[p,m,j] = i[p,m,k1[p,m,j]]
                sels = []
                for kk, ix in ((k1, i1), (k2, i2)):
                    eq = eqs.tile([128, 256], U16, tag="eq")
                    nc.vector.tensor_tensor(
                        out=eq[:, :].rearrange("p (m j k) -> p m j k", m=4, j=8),
                        in0=kk[:, :].rearrange("p (m j) -> p m j", m=4)
                            .unsqueeze(3).to_broadcast([128, 4, 8, 8]),
                        in1=iota8[:, :].unsqueeze(1).unsqueeze(1)
                            .to_broadcast([128, 4, 8, 8]),
                        op=OP.is_equal)
                    prod = eqs.tile([128, 256], U16, tag="prod")
                    nc.vector.tensor_tensor(
                        out=prod[:, :].rearrange("p (m j k) -> p m j k", m=4, j=8),
                        in0=eq[:, :].rearrange("p (m j k) -> p m j k", m=4, j=8),
                        in1=ix[:, :].rearrange("p (m k) -> p m k", m=4)
                            .unsqueeze(2).to_broadcast([128, 4, 8, 8]),
                        op=OP.mult)
                    sel = st2.tile([128, 32], U16, tag=f"sel{len(sels)}{tb}{H}",
                                   name=f"sel{len(sels)}{tb}{H}")
                    with nc.allow_low_precision(
                            reason="one-hot uint16 sum, values <= 255"):
                        nc.vector.reduce_sum(
                            out=sel[:],
                            in_=prod[:, :].rearrange("p (mj k) -> p mj k", k=8),
                            axis=X)
                    sels.append(sel)

                if use_dg:
                    # ---- wrap sub-key ids into dma_gather idx layout ----
                    selb = st2.tile([128, 64], BF16, tag=f"selb{tb}{H}",
                                    name=f"selb{tb}{H}")
                    nc.vector.tensor_copy(out=selb[:, 0:32], in_=sels[0][:])
                    nc.vector.tensor_copy(out=selb[:, 32:64], in_=sels[1][:])
                    w1 = st2.tile([128, 4, NCOL], U16, tag=f"w1{tb}{H}",
                                  name=f"w1{tb}{H}")
                    w2 = st2.tile([128, 4, NCOL], U16, tag=f"w2{tb}{H}",
                                  name=f"w2{tb}{H}")
                    for hi in range(8):
                        pf = pshfp.tile([128, 64], F32, tag="pshf")
                        nc.tensor.matmul(
                            out=pf[:], lhsT=shf_sb[:, hi, :], rhs=selb[:],
                            start=True, stop=True)
                        # W[q, oo, j*8+hi] = sel[16*hi + q%16, slot 8(4H+oo)+j]
                        for w, base in ((w1, 0), (w2, 32)):
                            nc.vector.tensor_copy(
                                out=w[:, :, hi:hi + 49:8],
                                in_=pf[:, base:base + 32].rearrange(
                                    "p (o j) -> p o j", o=4)[:, :, 0:7])
                    wx = st2.tile([128, 4, NCOL], U16, tag=f"wx{tb}{H}",
                                  name=f"wx{tb}{H}")
                    with nc.allow_low_precision(
                            reason="u16 expert-id packing, wraps by design"):
                        nc.vector.scalar_tensor_tensor(
                            out=wx[:], in0=w1[:], scalar=256, in1=w2[:],
                            op0=OP.mult, op1=OP.add)
                        nc.vector.tensor_scalar(
                            out=wx[:], in0=wx[:], scalar1=32768, scalar2=None,
                            op0=OP.bitwise_xor)
                        # pad column: idx 0 (>=0 as int16) at position 896
                        # defeats the trailing-negative trim
                        nc.vector.tensor_scalar(
                            out=wx[:, :, 56:57], in0=wx[:, :, 56:57],
                            scalar1=0, scalar2=None, op0=OP.mult)
                    widx[(tb, H)] = wx
                    # repair indices: the 8th slot of each head, gathered
                    # via plain single-offset indirect DMA (exact)
                    idxr = st2.tile([128, 4], U16, tag=f"idxr{tb}{H}",
                                    name=f"idxr{tb}{H}")
                    with nc.allow_low_precision(
                            reason="u16 expert-id packing, < 65536"):
                        nc.vector.tensor_scalar(
                            out=idxr[:], in0=sels[0][:, 7:32:8], scalar1=256,
                            scalar2=None, op0=OP.mult)
                        nc.vector.tensor_tensor(
                            out=idxr[:], in0=idxr[:], in1=sels[1][:, 7:32:8],
                            op=OP.add)
                    for oo in range(4):
                        ixg = st2.tile([128, 1], I32, tag=f"ixr{tb}{H}{oo}",
                                       name=f"ixr{tb}{H}{oo}")
                        nc.vector.tensor_copy(out=ixg[:],
                                              in_=idxr[:, oo:oo + 1])
                        idx32[(tb, 4 * H + oo)] = ixg
                else:
                    idx16 = st2.tile([128, 32], U16, tag=f"idx16{tb}{H}",
                                     name=f"idx16{tb}{H}")
                    nc.vector.tensor_scalar(
                        out=idx16[:], in0=sels[0][:], scalar1=256, scalar2=None,
                        op0=OP.mult)
                    nc.vector.tensor_tensor(
                        out=idx16[:], in0=idx16[:], in1=sels[1][:], op=OP.add)
                    for o in range(32):
                        s = 32 * H + o
                        ixg = st2.tile([128, 1], I32, tag=f"ixg{tb}_{s}",
                                       name=f"ixg{tb}_{s}")
                        nc.vector.tensor_copy(out=ixg[:],
                                              in_=idx16[:, o:o + 1])
                        idx32[(tb, s)] = ixg

                # ---- softmax over each head's top-8 ----
                rmax = st2.tile([128, 4], F32, tag=f"rmax{tb}{H}",
                                name=f"rmax{tb}{H}")
                nc.vector.reduce_max(
                    out=rmax[:], in_=v8[:, :].rearrange("p (m k) -> p m k", m=4),
                    axis=X)
                ex = st2.tile([128, 32], F32, tag=f"ex{tb}{H}",
                              name=f"ex{tb}{H}")
                nc.vector.tensor_tensor(
                    out=ex[:, :].rearrange("p (m k) -> p m k", m=4),
                    in0=v8[:, :].rearrange("p (m k) -> p m k", m=4),
                    in1=rmax[:, :].unsqueeze(2).to_broadcast([128, 4, 8]),
                    op=OP.subtract)
                nc.scalar.activation(
                    out=ex[:], in_=ex[:], func=mybir.ActivationFunctionType.Exp)
                rsum = st2.tile([128, 4], F32, tag=f"rsum{tb}{H}",
                                name=f"rsum{tb}{H}")
                nc.vector.reduce_sum(
                    out=rsum[:], in_=ex[:, :].rearrange("p (m k) -> p m k", m=4),
                    axis=X)
                rinv = st2.tile([128, 4], F32, tag=f"rinv{tb}{H}",
                                name=f"rinv{tb}{H}")
                nc.vector.reciprocal(out=rinv[:], in_=rsum[:])
                nc.vector.tensor_tensor(
                    out=ws[tb][:, 32 * H:32 * H + 32].rearrange(
                        "p (m k) -> p m k", m=4),
                    in0=ex[:, :].rearrange("p (m k) -> p m k", m=4),
                    in1=rinv[:, :].unsqueeze(2).to_broadcast([128, 4, 8]),
                    op=OP.mult)

            def issue_gathers(tb, H):
                if use_dg:
                    for oo in range(4):
                        o = 4 * H + oo
                        page = wbp.tile([128, 8, 1024], BF16, tag="wbpage",
                                        name=f"pg{tb}_{o}")
                        pages[(tb, o)] = page
                        nc.gpsimd.dma_gather(
                            out_ap=page[:],
                            in_ap=wb_d[32768:, :],
                            idxs_ap=widx[(tb, H)][:, oo, :].bitcast(I16),
                            num_idxs=897,
                            num_idxs_reg=897,
                            elem_size=1024,
                        )
                        pic = wbp.tile([128, 1024], BF16, tag="wbic",
                                       name=f"pic{tb}_{o}", bufs=4)
                        pages[(tb, o, "ic")] = pic
                        nc.gpsimd.indirect_dma_start(
                            out=pic[:], out_offset=None,
                            in_=wb_d[:, :],
                            in_offset=IndirectOffsetOnAxis(
                                ap=idx32[(tb, o)][:], axis=0),
                        )
                else:
                    for o in range(32):
                        s = 32 * H + o
                        page = wbp.tile([128, 1024], BF16, tag="wbpage",
                                        name=f"pg{tb}_{s}")
                        pages[(tb, s)] = page
                        nc.gpsimd.indirect_dma_start(
                            out=page[:], out_offset=None,
                            in_=wb_d[:, :],
                            in_offset=IndirectOffsetOnAxis(
                                ap=idx32[(tb, s)][:], axis=0),
                        )

            def get_page_slice(tb, grp, sidx, lo, hi):
                s = grp * 8 + sidx
                if use_dg:
                    if sidx == 7:
                        return pages[(tb, grp, "ic")][:, lo:hi]
                    return pages[(tb, grp)][:, sidx, lo:hi]
                return pages[(tb, s)][:, lo:hi]

            def compute(tb):
                inner = st2.tile([128, 64], F32, tag=f"inner{tb}",
                                 name=f"inner{tb}")
                va = st2.tile([128, 64], F32, tag=f"va{tb}", name=f"va{tb}")
                pacc = paccp.tile([128, D], F32, tag=f"pacc{tb}",
                                  name=f"pacc{tb}")
                for grp in range(8):
                    gs = slice(grp * 8, (grp + 1) * 8)
                    for sidx in range(8):
                        col = grp * 8 + sidx
                        wdrow = get_page_slice(tb, grp, sidx, 0, D)
                        scr = scrp.tile([128, D], BF16, tag="scr")
                        if sidx < STT_SPLIT:
                            nc.vector.scalar_tensor_tensor(
                                out=scr[:], in0=wdrow, scalar=1.0,
                                in1=xtok_bf[tb][:], op0=OP.mult, op1=OP.mult,
                                accum_out=inner[:, col:col + 1])
                        else:
                            nc.vector.tensor_tensor(
                                out=scr[:], in0=wdrow,
                                in1=xtok_bf[tb][:], op=OP.mult)
                            scr2 = scrp.tile([128, D], BF16, tag="scr2")
                            nc.scalar.activation(
                                out=scr2[:], in_=scr[:],
                                func=mybir.ActivationFunctionType.Copy,
                                accum_out=inner[:, col:col + 1])
                    rl8 = st2.tile([128, 8], F32, tag=f"rl{tb}", name=f"rl{tb}")
                    nc.scalar.activation(
                        out=rl8[:], in_=inner[:, gs],
                        func=mybir.ActivationFunctionType.Relu)
                    nc.vector.tensor_tensor(
                        out=va[:, gs], in0=rl8[:], in1=ws[tb][:, gs],
                        op=OP.mult)
                    # all 8 diag matrices of the group in one DVE op
                    dgrp = dgp.tile([128, 8, 128], BF16, tag="dgrp")
                    nc.vector.tensor_tensor(
                        out=dgrp[:],
                        in0=va[:, gs].unsqueeze(2).to_broadcast([128, 8, 128]),
                        in1=id01_sb[:].unsqueeze(1).to_broadcast([128, 8, 128]),
                        op=OP.mult)
                    for sidx in range(8):
                        col = grp * 8 + sidx
                        nc.tensor.matmul(
                            out=pacc[:], lhsT=dgrp[:, sidx, :],
                            rhs=get_page_slice(tb, grp, sidx, D, 2 * D),
                            start=(col == 0), stop=(col == 63))
                acc_sb = accp.tile([128, D], F32, tag=f"acc{tb}",
                                   name=f"acc{tb}")
                nc.vector.tensor_copy(out=acc_sb[:], in_=pacc[:])
                nc.sync.dma_start(
                    out=out_d[tb * 128:(tb + 1) * 128, :], in_=acc_sb[:])

            qproj(0)
            scores_stage(0, 0)
            scores_stage(0, 1)
            qproj(1)
            resolve_half(0, 0)
            issue_gathers(0, 0)
            resolve_half(0, 1)
            issue_gathers(0, 1)
            scores_stage(1, 0)
            scores_stage(1, 1)
            resolve_half(1, 0)
            issue_gathers(1, 0)
            resolve_half(1, 1)
            issue_gathers(1, 1)
            compute(0)
            compute(1)

    nc.compile()
    return nc


_NC_CACHE = None
VARIANT = {}


def _get_nc():
    global _NC_CACHE
    if _NC_CACHE is None:
        _NC_CACHE = build_nc(**VARIANT)
    return _NC_CACHE


def _prep_in_maps(inputs):
    q = np.ascontiguousarray(np.asarray(inputs["queries"], dtype=np.float32))
    Wq = np.ascontiguousarray(np.asarray(inputs["Wq"], dtype=np.float32))
    bq = np.asarray(inputs["bq"], dtype=np.float32)
    keys = np.asarray(inputs["keys"], dtype=np.float32)
    wd = np.asarray(inputs["w_down"], dtype=np.float32)
    wu = np.asarray(inputs["w_up"], dtype=np.float32)
    import ml_dtypes
    wb = np.ascontiguousarray(
        np.concatenate([wd, wu], axis=1).astype(ml_dtypes.bfloat16))
    id01 = np.eye(128, dtype=np.float32).astype(ml_dtypes.bfloat16)

    x = q.reshape(B, D)
    # bqp[p, m] = bq[m*128 + p]
    bqp = np.ascontiguousarray(bq.reshape(N_HEADS, D_KEYS).T)
    # block-diagonal keys: kbd[p, m, 0, n] = keys[m, 0, n, p] for p < 64,
    # kbd[p, m, 1, n] = keys[m, 1, n, p-64] for p >= 64, zero elsewhere.
    kbd = np.zeros((D_KEYS, N_HEADS, 2, N_KEYS), np.float32)
    kbd[:HALF, :, 0, :] = keys[:, 0].transpose(2, 0, 1)
    kbd[HALF:, :, 1, :] = keys[:, 1].transpose(2, 0, 1)
    kbd = np.ascontiguousarray(kbd)
    # shuffle/replicate matrices: shf[p, hi, f] = 1 iff p == 16*hi + f%16
    shf = np.zeros((128, 8, 128), np.float32)
    for hi in range(8):
        for f in range(128):
            shf[16 * hi + f % 16, hi, f] = 1.0
    shf = np.ascontiguousarray(shf.astype(ml_dtypes.bfloat16))
    iota8 = np.broadcast_to(np.arange(8, dtype=np.uint16), (128, 8))
    iota8 = np.ascontiguousarray(iota8)

    in_maps = []
    for c in range(N_CORES):
        xc = x[c * BC:(c + 1) * BC]
        in_maps.append({
            "xtokb": np.ascontiguousarray(xc.astype(ml_dtypes.bfloat16)),
            "xt": np.ascontiguousarray(xc.T),
            "wq": Wq,
            "bqp": bqp,
            "kbd": kbd,
            "wb": wb,
            "id01": id01,
            "shf": shf,
            "iota8": iota8,
        })
    return in_maps


def run(inputs, trace=False):
    """Run on 8 NeuronCores; returns (out [2,1024,512], BassKernelResults)."""
    nc = _get_nc()
    in_maps = _prep_in_maps(inputs)
    res = run_bass_kernel_spmd(
        nc, in_maps, core_ids=list(range(N_CORES)), trace=trace)
    out = np.concatenate(
        [res.results[c]["out"] for c in range(N_CORES)], axis=0)
    return out.reshape(2, 1024, D), res


def kernel(**inputs) -> np.ndarray:
    out, _ = run(inputs, trace=False)
    return out


# revision 22
# speedup vs baseline: 1.1237x; 1.1237x over previous
"""PEER / product-key MoE routing kernel for Trainium2 (8 NeuronCores).

Strategy: data-parallel over tokens. Each of the 8 cores gets 256 of the
2048 tokens plus a full replica of the expert tables in its DRAM. Routing
(q projection, product-key scores, two-stage top-8), expert-row gathers,
and the PEER combine all run on-device. No collectives are needed; the
host only slices/packs inputs and concatenates the per-core outputs.

Per-core pipeline (v5):
  PE:  qT = Wq^T @ x^T with 128-wide feature tiles (fp32, exact);
       per-head scores via ONE matmul against block-diagonal keys.
  DVE: top-8 of each 256-score PSUM half via max8/max_index (exact),
       then top-8 of the 8x8 combo sums; winners' sub-key ids resolved
       with an is_equal one-hot reduction.
  PE+DVE: sub-key ids are shuffled into dma_gather's wrapped index
       layout ([16 partitions, n/16] int16) with 8 tiny permutation
       matmuls per token block (values <= 255, bf16-exact), then
       combined to expert ids and biased by -32768 (bitwise xor) so the
       17-bit id range fits int16 (the gather ucode uses a signed MAC;
       the source AP is pre-offset by +32768 rows to compensate).
  GPSIMD: ONE dma_gather per 8-slot group (1024 expert-row-pairs + 1
       pad index, 2 MB) - vectorized descriptor emission replaces 64
       per-slot indirect DMAs whose fixed costs starved the SDMA
       engines (the old path burned 150us of GpSimd time).
  DVE/ACT: inner products as bf16 multiply + free-dim-sum, split
       between the fused DVE scalar_tensor_tensor path and the
       DVE-mult + ACT-accum path to balance the two engines.
  PE:  combine as PSUM-accumulated diag(vals) @ w_up_row matmuls; the
       8 diag matrices of a group are built in one DVE op.

Routing is computed entirely in fp32, so expert selection matches the
fp32 reference exactly; only the expert tables are bf16 (rel err ~4e-3).
"""

import numpy as np

import concourse.bass as bass
import concourse.mybir as mybir
from concourse import bacc
from concourse import library_config
from concourse.bass import IndirectOffsetOnAxis
from concourse.tile import TileContext
from concourse.bass_utils import run_bass_kernel_spmd

N_CORES = 8
N_HEADS = 8
D_KEYS = 128
HALF = 64
N_KEYS = 256
TOP_K = 8
D = 512
B = 2048           # total tokens
BC = B // N_CORES  # tokens per core (256)
TB = BC // 128     # token blocks per core (2)
# dma_gather is limited to 1024 indices per op and trims trailing
# negative (biased) indices. Each op covers one head's first 7 slots
# (n = 897: 896 real + 1 pad index 0 at position 896 defeats the trim;
# mid-list negatives are fine - validity is position-based). The 8th
# slot of each head is gathered with a plain single-offset indirect
# DMA, which is exact and cheap at 128 descriptors.
NOPS = 8           # dma_gather ops per token block (one per head)
NCOL = 57          # wrapped idx columns per op (ceil(897/16))
F32 = mybir.dt.float32
U16 = mybir.dt.uint16
I16 = mybir.dt.int16
I32 = mybir.dt.int32
BF16 = mybir.dt.bfloat16
X = mybir.AxisListType.X
OP = mybir.AluOpType

# inner-product engine split within each 8-slot group:
# slots < STT_SPLIT use the fused DVE scalar_tensor_tensor, the rest use
# DVE-mult + ACT-accum (balances DVE vs Scalar engine load)
STT_SPLIT = 1


def build_nc(use_dg=True):
    nc = bacc.Bacc("TRN2", target_bir_lowering=False)

    xtokb_d = nc.dram_tensor("xtokb", [BC, D], BF16, kind="ExternalInput")
    xt_d = nc.dram_tensor("xt", [D, BC], F32, kind="ExternalInput")
    wq_d = nc.dram_tensor("wq", [D, N_HEADS * D_KEYS], F32, kind="ExternalInput")
    bqp_d = nc.dram_tensor("bqp", [D_KEYS, N_HEADS], F32, kind="ExternalInput")
    kbd_d = nc.dram_tensor("kbd", [D_KEYS, N_HEADS, 2, N_KEYS], F32,
                           kind="ExternalInput")
    wb_d = nc.dram_tensor("wb", [N_KEYS * N_KEYS, 2 * D], BF16,
                          kind="ExternalInput")
    id01_d = nc.dram_tensor("id01", [128, 128], BF16, kind="ExternalInput")
    shf_d = nc.dram_tensor("shf", [128, 8, 128], BF16, kind="ExternalInput")
    iota8_d = nc.dram_tensor("iota8", [128, 8], U16, kind="ExternalInput")
    out_d = nc.dram_tensor("out", [BC, D], F32, kind="ExternalOutput")

    with TileContext(nc) as tc:
        with (
            tc.tile_pool(name="const", bufs=1) as cpool,
            tc.tile_pool(name="psq", bufs=2, space="PSUM") as psq,
            tc.tile_pool(name="pss", bufs=2, space="PSUM") as pss,
            tc.tile_pool(name="pshf", bufs=2, space="PSUM") as pshfp,
            tc.tile_pool(name="st2", bufs=1) as st2,
            tc.tile_pool(name="eqs", bufs=2) as eqs,
            tc.tile_pool(name="wbp", bufs=5 if use_dg else 32) as wbp,
            tc.tile_pool(name="scr", bufs=4) as scrp,
            tc.tile_pool(name="dgp", bufs=4) as dgp,
            tc.tile_pool(name="pacc", bufs=1, space="PSUM") as paccp,
            tc.tile_pool(name="accp", bufs=2) as accp,
        ):
            if use_dg:
                nc.gpsimd.load_library(library_config.mlp)
            # ---- constant loads (ordered by first use) ----
            wq_sb = []
            xt_sb = []
            for k in range(4):
                t2 = cpool.tile([128, BC], F32, tag=f"xt{k}")
                nc.sync.dma_start(out=t2[:], in_=xt_d[k * 128:(k + 1) * 128, :])
                xt_sb.append(t2)
                t = cpool.tile([128, N_HEADS * D_KEYS], F32, tag=f"wq{k}")
                nc.sync.dma_start(out=t[:], in_=wq_d[k * 128:(k + 1) * 128, :])
                wq_sb.append(t)
            bqp_sb = cpool.tile([D_KEYS, N_HEADS], F32, tag="bqp")
            nc.sync.dma_start(out=bqp_sb[:], in_=bqp_d[:, :])
            kbd_sb = cpool.tile([D_KEYS, N_HEADS, 2, N_KEYS], F32, tag="kbd")
            nc.sync.dma_start(out=kbd_sb[:], in_=kbd_d[:, :, :, :])
            iota8 = cpool.tile([128, 8], U16, tag="iota8")
            nc.sync.dma_start(out=iota8[:], in_=iota8_d[:, :])
            shf_sb = cpool.tile([128, 8, 128], BF16, tag="shf")
            nc.sync.dma_start(out=shf_sb[:], in_=shf_d[:, :, :])
            xtok_bf = []
            for tb in range(TB):
                tb16 = cpool.tile([128, D], BF16, tag=f"xtokb{tb}")
                nc.sync.dma_start(out=tb16[:], in_=xtokb_d[tb * 128:(tb + 1) * 128, :])
                xtok_bf.append(tb16)
            id01_sb = cpool.tile([128, 128], BF16, tag="id01")
            nc.sync.dma_start(out=id01_sb[:], in_=id01_d[:, :])

            qts = [cpool.tile([D_KEYS, N_HEADS, 128], F32, tag=f"qt{tb}",
                              name=f"qt{tb}") for tb in range(TB)]
            widx = {}    # (tb, H) -> wrapped-index tile [128, 4, NCOL] U16
            idx32 = {}   # (tb, op) -> [128, 1] I32 offsets
            ws = {tb: st2.tile([128, 64], F32, tag=f"w8{tb}", name=f"w8{tb}")
                  for tb in range(TB)}
            pages = {}

            def qproj_half(tb, H):
                tsl = slice(tb * 128, (tb + 1) * 128)
                qt = qts[tb]
                # ---- qT per head: [feature-in-head, token] (fp32, exact) ----
                for m in range(4 * H, 4 * H + 4):
                    ps = psq.tile([128, 128], F32, tag="psq")
                    for k in range(4):
                        nc.tensor.matmul(
                            out=ps[:],
                            lhsT=wq_sb[k][:, m * 128:(m + 1) * 128],
                            rhs=xt_sb[k][:, tsl],
                            start=(k == 0),
                            stop=(k == 3),
                        )
                    nc.vector.tensor_scalar(
                        out=qt[:, m, :], in0=ps[:],
                        scalar1=bqp_sb[:, m:m + 1], scalar2=None, op0=OP.add,
                    )

            stage1 = {}

            def scores_stage(tb, H):
                # scores + stage-1 top8 for heads 4H..4H+3 (exact)
                qt = qts[tb]
                s1t = st2.tile([128, 32], F32, tag=f"s1t{tb}{H}",
                               name=f"s1t{tb}{H}")
                s2t = st2.tile([128, 32], F32, tag=f"s2t{tb}{H}",
                               name=f"s2t{tb}{H}")
                i1 = st2.tile([128, 32], U16, tag=f"i1{tb}{H}",
                              name=f"i1{tb}{H}")
                i2 = st2.tile([128, 32], U16, tag=f"i2{tb}{H}",
                              name=f"i2{tb}{H}")
                for mm in range(4):
                    m = 4 * H + mm
                    ps2 = pss.tile([128, 2, N_KEYS], F32, tag="pss")
                    nc.tensor.matmul(
                        out=ps2[:, :, :].rearrange("p a b -> p (a b)"),
                        lhsT=qt[:, m, :],
                        rhs=kbd_sb[:, m, :, :].rearrange("p a b -> p (a b)"),
                        start=True, stop=True,
                    )
                    for half, (st_, ix) in enumerate(((s1t, i1), (s2t, i2))):
                        nc.vector.max(out=st_[:, mm * 8:(mm + 1) * 8],
                                      in_=ps2[:, half, :])
                        nc.vector.max_index(
                            out=ix[:, mm * 8:(mm + 1) * 8],
                            in_max=st_[:, mm * 8:(mm + 1) * 8],
                            in_values=ps2[:, half, :],
                        )
                stage1[(tb, H)] = (s1t, s2t, i1, i2)

            def resolve_half(tb, H):
                s1t, s2t, i1, i2 = stage1[(tb, H)]
                # ---- stage-2: 8x8 combo scores, top8 of 64 per head ----
                cs = st2.tile([128, 256], F32, tag=f"cs{tb}{H}",
                              name=f"cs{tb}{H}")
                for mm in range(4):
                    nc.vector.tensor_tensor(
                        out=cs[:, mm * 64:(mm + 1) * 64].rearrange(
                            "p (a b) -> p a b", a=8),
                        in0=s1t[:, mm * 8:(mm + 1) * 8].unsqueeze(2)
                            .to_broadcast([128, 8, 8]),
                        in1=s2t[:, mm * 8:(mm + 1) * 8].unsqueeze(1)
                            .to_broadcast([128, 8, 8]),
                        op=OP.add,
                    )
                v8 = st2.tile([128, 32], F32, tag=f"v8{tb}{H}",
                              name=f"v8{tb}{H}")
                n8 = st2.tile([128, 32], U16, tag=f"n8{tb}{H}",
                              name=f"n8{tb}{H}")
                for mm in range(4):
                    nc.vector.max(out=v8[:, mm * 8:(mm + 1) * 8],
                                  in_=cs[:, mm * 64:(mm + 1) * 64])
                    nc.vector.max_index(
                        out=n8[:, mm * 8:(mm + 1) * 8],
                        in_max=v8[:, mm * 8:(mm + 1) * 8],
                        in_values=cs[:, mm * 64:(mm + 1) * 64])
                k1 = st2.tile([128, 32], U16, tag=f"k1{tb}{H}",
                              name=f"k1{tb}{H}")
                nc.vector.tensor_scalar(
                    out=k1[:], in0=n8[:], scalar1=3, scalar2=None,
                    op0=OP.logical_shift_right)
                k2 = st2.tile([128, 32], U16, tag=f"k2{tb}{H}",
                              name=f"k2{tb}{H}")
                nc.vector.tensor_scalar(
                    out=k2[:], in0=n8[:], scalar1=7, scalar2=None,
                    op0=OP.bitwise_and)

                # resolve winners' sub-key ids: isel[p,m,j] = i[p,m,k1[p,m,j]]
                sels = []
                for kk, ix in ((k1, i1), (k2, i2)):
                    eq = eqs.tile([128, 256], U16, tag="eq")
                    nc.vector.tensor_tensor(
                        out=eq[:, :].rearrange("p (m j k) -> p m j k", m=4, j=8),
                        in0=kk[:, :].rearrange("p (m j) -> p m j", m=4)
                            .unsqueeze(3).to_broadcast([128, 4, 8, 8]),
                        in1=iota8[:, :].unsqueeze(1).unsqueeze(1)
                            .to_broadcast([128, 4, 8, 8]),
                        op=OP.is_equal)
                    prod = eqs.tile([128, 256], U16, tag="prod")
                    nc.vector.tensor_tensor(
                        out=prod[:, :].rearrange("p (m j k) -> p m j k", m=4, j=8),
                        in0=eq[:, :].rearrange("p (m j k) -> p m j k", m=4, j=8),
                        in1=ix[:, :].rearrange("p (m k) -> p m k", m=4)
                            .unsqueeze(2).to_broadcast([128, 4, 8, 8]),
                        op=OP.mult)
                    sel = st2.tile([128, 32], U16, tag=f"sel{len(sels)}{tb}{H}",
                                   name=f"sel{len(sels)}{tb}{H}")
                    with nc.allow_low_precision(
                            reason="one-hot uint16 sum, values <= 255"):
                        nc.vector.reduce_sum(
                            out=sel[:],
                            in_=prod[:, :].rearrange("p (mj k) -> p mj k", k=8),
                            axis=X)
                    sels.append(sel)

                if use_dg:
                    # ---- wrap sub-key ids into dma_gather idx layout ----
                    selb = st2.tile([128, 64], BF16, tag=f"selb{tb}{H}",
                                    name=f"selb{tb}{H}")
                    nc.vector.tensor_copy(out=selb[:, 0:32], in_=sels[0][:])
                    nc.vector.tensor_copy(out=selb[:, 32:64], in_=sels[1][:])
                    w1 = st2.tile([128, 4, NCOL], U16, tag=f"w1{tb}{H}",
                                  name=f"w1{tb}{H}")
                    w2 = st2.tile([128, 4, NCOL], U16, tag=f"w2{tb}{H}",
                                  name=f"w2{tb}{H}")
                    for hi in range(8):
                        pf = pshfp.tile([128, 64], F32, tag="pshf")
                        nc.tensor.matmul(
                            out=pf[:], lhsT=shf_sb[:, hi, :], rhs=selb[:],
                            start=True, stop=True)
                        # W[q, oo, j*8+hi] = sel[16*hi + q%16, slot 8(4H+oo)+j]
                        for w, base in ((w1, 0), (w2, 32)):
                            nc.vector.tensor_copy(
                                out=w[:, :, hi:hi + 49:8],
                                in_=pf[:, base:base + 32].rearrange(
                                    "p (o j) -> p o j", o=4)[:, :, 0:7])
                    wx = st2.tile([128, 4, NCOL], U16, tag=f"wx{tb}{H}",
                                  name=f"wx{tb}{H}")
                    with nc.allow_low_precision(
                            reason="u16 expert-id packing, wraps by design"):
                        nc.vector.scalar_tensor_tensor(
                            out=wx[:], in0=w1[:], scalar=256, in1=w2[:],
                            op0=OP.mult, op1=OP.add)
                        nc.vector.tensor_scalar(
                            out=wx[:], in0=wx[:], scalar1=32768, scalar2=None,
                            op0=OP.bitwise_xor)
                        # pad column: idx 0 (>=0 as int16) at position 896
                        # defeats the trailing-negative trim
                        nc.vector.tensor_scalar(
                            out=wx[:, :, 56:57], in0=wx[:, :, 56:57],
                            scalar1=0, scalar2=None, op0=OP.mult)
                    widx[(tb, H)] = wx
                    # repair indices: the 8th slot of each head, gathered
                    # via plain single-offset indirect DMA (exact)
                    idxr = st2.tile([128, 4], U16, tag=f"idxr{tb}{H}",
                                    name=f"idxr{tb}{H}")
                    with nc.allow_low_precision(
                            reason="u16 expert-id packing, < 65536"):
                        nc.vector.tensor_scalar(
                            out=idxr[:], in0=sels[0][:, 7:32:8], scalar1=256,
                            scalar2=None, op0=OP.mult)
                        nc.vector.tensor_tensor(
                            out=idxr[:], in0=idxr[:], in1=sels[1][:, 7:32:8],
                            op=OP.add)
                    for oo in range(4):
                        ixg = st2.tile([128, 1], I32, tag=f"ixr{tb}{H}{oo}",
                                       name=f"ixr{tb}{H}{oo}")
                        nc.vector.tensor_copy(out=ixg[:],
                                              in_=idxr[:, oo:oo + 1])
                        idx32[(tb, 4 * H + oo)] = ixg
                else:
                    idx16 = st2.tile([128, 32], U16, tag=f"idx16{tb}{H}",
                                     name=f"idx16{tb}{H}")
                    nc.vector.tensor_scalar(
                        out=idx16[:], in0=sels[0][:], scalar1=256, scalar2=None,
                        op0=OP.mult)
                    nc.vector.tensor_tensor(
                        out=idx16[:], in0=idx16[:], in1=sels[1][:], op=OP.add)
                    for o in range(32):
                        s = 32 * H + o
                        ixg = st2.tile([128, 1], I32, tag=f"ixg{tb}_{s}",
                                       name=f"ixg{tb}_{s}")
                        nc.vector.tensor_copy(out=ixg[:],
                                              in_=idx16[:, o:o + 1])
                        idx32[(tb, s)] = ixg

                # ---- softmax over each head's top-8 ----
                rmax = st2.tile([128, 4], F32, tag=f"rmax{tb}{H}",
                                name=f"rmax{tb}{H}")
                nc.vector.reduce_max(
                    out=rmax[:], in_=v8[:, :].rearrange("p (m k) -> p m k", m=4),
                    axis=X)
                ex = st2.tile([128, 32], F32, tag=f"ex{tb}{H}",
                              name=f"ex{tb}{H}")
                nc.vector.tensor_tensor(
                    out=ex[:, :].rearrange("p (m k) -> p m k", m=4),
                    in0=v8[:, :].rearrange("p (m k) -> p m k", m=4),
                    in1=rmax[:, :].unsqueeze(2).to_broadcast([128, 4, 8]),
                    op=OP.subtract)
                nc.scalar.activation(
                    out=ex[:], in_=ex[:], func=mybir.ActivationFunctionType.Exp)
                rsum = st2.tile([128, 4], F32, tag=f"rsum{tb}{H}",
                                name=f"rsum{tb}{H}")
                nc.vector.reduce_sum(
                    out=rsum[:], in_=ex[:, :].rearrange("p (m k) -> p m k", m=4),
                    axis=X)
                rinv = st2.tile([128, 4], F32, tag=f"rinv{tb}{H}",
                                name=f"rinv{tb}{H}")
                nc.vector.reciprocal(out=rinv[:], in_=rsum[:])
                nc.vector.tensor_tensor(
                    out=ws[tb][:, 32 * H:32 * H + 32].rearrange(
                        "p (m k) -> p m k", m=4),
                    in0=ex[:, :].rearrange("p (m k) -> p m k", m=4),
                    in1=rinv[:, :].unsqueeze(2).to_broadcast([128, 4, 8]),
                    op=OP.mult)

            def issue_gathers(tb, H):
                if use_dg:
                    for oo in range(4):
                        o = 4 * H + oo
                        page = wbp.tile([128, 8, 1024], BF16, tag="wbpage",
                                        name=f"pg{tb}_{o}")
                        pages[(tb, o)] = page
                        nc.gpsimd.dma_gather(
                            out_ap=page[:],
                            in_ap=wb_d[32768:, :],
                            idxs_ap=widx[(tb, H)][:, oo, :].bitcast(I16),
                            num_idxs=897,
                            num_idxs_reg=897,
                            elem_size=1024,
                        )
                        pic = wbp.tile([128, 1024], BF16, tag="wbic",
                                       name=f"pic{tb}_{o}", bufs=8)
                        pages[(tb, o, "ic")] = pic
                        nc.gpsimd.indirect_dma_start(
                            out=pic[:], out_offset=None,
                            in_=wb_d[:, :],
                            in_offset=IndirectOffsetOnAxis(
                                ap=idx32[(tb, o)][:], axis=0),
                        )
                else:
                    for o in range(32):
                        s = 32 * H + o
                        page = wbp.tile([128, 1024], BF16, tag="wbpage",
                                        name=f"pg{tb}_{s}")
                        pages[(tb, s)] = page
                        nc.gpsimd.indirect_dma_start(
                            out=page[:], out_offset=None,
                            in_=wb_d[:, :],
                            in_offset=IndirectOffsetOnAxis(
                                ap=idx32[(tb, s)][:], axis=0),
                        )

            def get_page_slice(tb, grp, sidx, lo, hi):
                s = grp * 8 + sidx
                if use_dg:
                    if sidx == 7:
                        return pages[(tb, grp, "ic")][:, lo:hi]
                    return pages[(tb, grp)][:, sidx, lo:hi]
                return pages[(tb, s)][:, lo:hi]

            inner_t = {}
            va_t = {}
            pacc_t = {}

            def compute_half(tb, H):
                if H == 0:
                    inner_t[tb] = st2.tile([128, 64], F32, tag=f"inner{tb}",
                                           name=f"inner{tb}")
                    va_t[tb] = st2.tile([128, 64], F32, tag=f"va{tb}",
                                        name=f"va{tb}")
                    pacc_t[tb] = paccp.tile([128, D], F32, tag=f"pacc{tb}",
                                            name=f"pacc{tb}")
                inner, va, pacc = inner_t[tb], va_t[tb], pacc_t[tb]
                for grp in range(4 * H, 4 * H + 4):
                    gs = slice(grp * 8, (grp + 1) * 8)
                    for sidx in range(8):
                        col = grp * 8 + sidx
                        wdrow = get_page_slice(tb, grp, sidx, 0, D)
                        scr = scrp.tile([128, D], BF16, tag="scr")
                        if sidx < STT_SPLIT:
                            nc.vector.scalar_tensor_tensor(
                                out=scr[:], in0=wdrow, scalar=1.0,
                                in1=xtok_bf[tb][:], op0=OP.mult, op1=OP.mult,
                                accum_out=inner[:, col:col + 1])
                        else:
                            nc.vector.tensor_tensor(
                                out=scr[:], in0=wdrow,
                                in1=xtok_bf[tb][:], op=OP.mult)
                            scr2 = scrp.tile([128, D], BF16, tag="scr2")
                            nc.scalar.activation(
                                out=scr2[:], in_=scr[:],
                                func=mybir.ActivationFunctionType.Copy,
                                accum_out=inner[:, col:col + 1])
                    rl8 = st2.tile([128, 8], F32, tag=f"rl{tb}", name=f"rl{tb}")
                    nc.scalar.activation(
                        out=rl8[:], in_=inner[:, gs],
                        func=mybir.ActivationFunctionType.Relu)
                    nc.vector.tensor_tensor(
                        out=va[:, gs], in0=rl8[:], in1=ws[tb][:, gs],
                        op=OP.mult)
                    # all 8 diag matrices of the group in one DVE op
                    dgrp = dgp.tile([128, 8, 128], BF16, tag="dgrp")
                    nc.vector.tensor_tensor(
                        out=dgrp[:],
                        in0=va[:, gs].unsqueeze(2).to_broadcast([128, 8, 128]),
                        in1=id01_sb[:].unsqueeze(1).to_broadcast([128, 8, 128]),
                        op=OP.mult)
                    for sidx in range(8):
                        col = grp * 8 + sidx
                        nc.tensor.matmul(
                            out=pacc[:], lhsT=dgrp[:, sidx, :],
                            rhs=get_page_slice(tb, grp, sidx, D, 2 * D),
                            start=(col == 0), stop=(col == 63))
                if H == 1:
                    acc_sb = accp.tile([128, D], F32, tag=f"acc{tb}",
                                       name=f"acc{tb}")
                    nc.vector.tensor_copy(out=acc_sb[:], in_=pacc[:])
                    nc.sync.dma_start(
                        out=out_d[tb * 128:(tb + 1) * 128, :], in_=acc_sb[:])

            qproj_half(0, 0)
            scores_stage(0, 0)
            qproj_half(0, 1)
            resolve_half(0, 0)
            issue_gathers(0, 0)
            scores_stage(0, 1)
            resolve_half(0, 1)
            issue_gathers(0, 1)
            qproj_half(1, 0)
            scores_stage(1, 0)
            qproj_half(1, 1)
            resolve_half(1, 0)
            compute_half(0, 0)
            issue_gathers(1, 0)
            scores_stage(1, 1)
            resolve_half(1, 1)
            compute_half(0, 1)
            issue_gathers(1, 1)
            compute_half(1, 0)
            compute_half(1, 1)

    nc.compile()
    return nc


_NC_CACHE = None
VARIANT = {}


def _get_nc():
    global _NC_CACHE
    if _NC_CACHE is None:
        _NC_CACHE = build_nc(**VARIANT)
    return _NC_CACHE


def _prep_in_maps(inputs):
    q = np.ascontiguousarray(np.asarray(inputs["queries"], dtype=np.float32))
    Wq = np.ascontiguousarray(np.asarray(inputs["Wq"], dtype=np.float32))
    bq = np.asarray(inputs["bq"], dtype=np.float32)
    keys = np.asarray(inputs["keys"], dtype=np.float32)
    wd = np.asarray(inputs["w_down"], dtype=np.float32)
    wu = np.asarray(inputs["w_up"], dtype=np.float32)
    import ml_dtypes
    wb = np.ascontiguousarray(
        np.concatenate([wd, wu], axis=1).astype(ml_dtypes.bfloat16))
    id01 = np.eye(128, dtype=np.float32).astype(ml_dtypes.bfloat16)

    x = q.reshape(B, D)
    # bqp[p, m] = bq[m*128 + p]
    bqp = np.ascontiguousarray(bq.reshape(N_HEADS, D_KEYS).T)
    # block-diagonal keys: kbd[p, m, 0, n] = keys[m, 0, n, p] for p < 64,
    # kbd[p, m, 1, n] = keys[m, 1, n, p-64] for p >= 64, zero elsewhere.
    kbd = np.zeros((D_KEYS, N_HEADS, 2, N_KEYS), np.float32)
    kbd[:HALF, :, 0, :] = keys[:, 0].transpose(2, 0, 1)
    kbd[HALF:, :, 1, :] = keys[:, 1].transpose(2, 0, 1)
    kbd = np.ascontiguousarray(kbd)
    # shuffle/replicate matrices: shf[p, hi, f] = 1 iff p == 16*hi + f%16
    shf = np.zeros((128, 8, 128), np.float32)
    for hi in range(8):
        for f in range(128):
            shf[16 * hi + f % 16, hi, f] = 1.0
    shf = np.ascontiguousarray(shf.astype(ml_dtypes.bfloat16))
    iota8 = np.broadcast_to(np.arange(8, dtype=np.uint16), (128, 8))
    iota8 = np.ascontiguousarray(iota8)

    in_maps = []
    for c in range(N_CORES):
        xc = x[c * BC:(c + 1) * BC]
        in_maps.append({
            "xtokb": np.ascontiguousarray(xc.astype(ml_dtypes.bfloat16)),
            "xt": np.ascontiguousarray(xc.T),
            "wq": Wq,
            "bqp": bqp,
            "kbd": kbd,
            "wb": wb,
            "id01": id01,
            "shf": shf,
            "iota8": iota8,
        })
    return in_maps


def run(inputs, trace=False):
    """Run on 8 NeuronCores; returns (out [2,1024,512], BassKernelResults)."""
    nc = _get_nc()
    in_maps = _prep_in_maps(inputs)
    res = run_bass_kernel_spmd(
        nc, in_maps, core_ids=list(range(N_CORES)), trace=trace)
    out = np.concatenate(
        [res.results[c]["out"] for c in range(N_CORES)], axis=0)
    return out.reshape(2, 1024, D), res


def kernel(**inputs) -> np.ndarray:
    out, _ = run(inputs, trace=False)
    return out


# revision 23
# speedup vs baseline: 1.1487x; 1.0223x over previous
"""PEER / product-key MoE routing kernel for Trainium2 (8 NeuronCores).

Strategy: data-parallel over tokens. Each of the 8 cores gets 256 of the
2048 tokens plus a full replica of the expert tables in its DRAM. Routing
(q projection, product-key scores, two-stage top-8), expert-row gathers,
and the PEER combine all run on-device. No collectives are needed; the
host only slices/packs inputs and concatenates the per-core outputs.

Per-core pipeline (v5):
  PE:  qT = Wq^T @ x^T with 128-wide feature tiles (fp32, exact);
       per-head scores via ONE matmul against block-diagonal keys.
  DVE: top-8 of each 256-score PSUM half via max8/max_index (exact),
       then top-8 of the 8x8 combo sums; winners' sub-key ids resolved
       with an is_equal one-hot reduction.
  PE+DVE: sub-key ids are shuffled into dma_gather's wrapped index
       layout ([16 partitions, n/16] int16) with 8 tiny permutation
       matmuls per token block (values <= 255, bf16-exact), then
       combined to expert ids and biased by -32768 (bitwise xor) so the
       17-bit id range fits int16 (the gather ucode uses a signed MAC;
       the source AP is pre-offset by +32768 rows to compensate).
  GPSIMD: ONE dma_gather per 8-slot group (1024 expert-row-pairs + 1
       pad index, 2 MB) - vectorized descriptor emission replaces 64
       per-slot indirect DMAs whose fixed costs starved the SDMA
       engines (the old path burned 150us of GpSimd time).
  DVE/ACT: inner products as bf16 multiply + free-dim-sum, split
       between the fused DVE scalar_tensor_tensor path and the
       DVE-mult + ACT-accum path to balance the two engines.
  PE:  combine as PSUM-accumulated diag(vals) @ w_up_row matmuls; the
       8 diag matrices of a group are built in one DVE op.

Routing is computed entirely in fp32, so expert selection matches the
fp32 reference exactly; only the expert tables are bf16 (rel err ~4e-3).
"""

import numpy as np

import concourse.bass as bass
import concourse.mybir as mybir
from concourse import bacc
from concourse import library_config
from concourse.bass import IndirectOffsetOnAxis
from concourse.tile import TileContext
from concourse.bass_utils import run_bass_kernel_spmd

N_CORES = 8
N_HEADS = 8
D_KEYS = 128
HALF = 64
N_KEYS = 256
TOP_K = 8
D = 512
B = 2048           # total tokens
BC = B // N_CORES  # tokens per core (256)
TB = BC // 128     # token blocks per core (2)
# dma_gather is limited to 1024 indices per op and trims trailing
# negative (biased) indices. Each op covers one head's first 7 slots
# (n = 897: 896 real + 1 pad index 0 at position 896 defeats the trim;
# mid-list negatives are fine - validity is position-based). The 8th
# slot of each head is gathered with a plain single-offset indirect
# DMA, which is exact and cheap at 128 descriptors.
NOPS = 8           # dma_gather ops per token block (one per head)
NCOL = 57          # wrapped idx columns per op (ceil(897/16))
F32 = mybir.dt.float32
U16 = mybir.dt.uint16
I16 = mybir.dt.int16
I32 = mybir.dt.int32
BF16 = mybir.dt.bfloat16
X = mybir.AxisListType.X
OP = mybir.AluOpType

# inner-product engine split within each 8-slot group: slots below the
# split use the fused DVE scalar_tensor_tensor, the rest use DVE-mult +
# ACT-accum. The bulk goes to ACT (DVE is the more contended engine);
# the final group of the kernel splits 4/4 to shorten the drain tail.
def stt_split(tb, grp):
    return 4 if (tb == TB - 1 and grp == 7) else 0


def build_nc(use_dg=True):
    nc = bacc.Bacc("TRN2", target_bir_lowering=False)

    xtokb_d = nc.dram_tensor("xtokb", [BC, D], BF16, kind="ExternalInput")
    xt_d = nc.dram_tensor("xt", [D, BC], F32, kind="ExternalInput")
    wq_d = nc.dram_tensor("wq", [D, N_HEADS * D_KEYS], F32, kind="ExternalInput")
    bqp_d = nc.dram_tensor("bqp", [D_KEYS, N_HEADS], F32, kind="ExternalInput")
    kbd_d = nc.dram_tensor("kbd", [D_KEYS, N_HEADS, 2, N_KEYS], F32,
                           kind="ExternalInput")
    wb_d = nc.dram_tensor("wb", [N_KEYS * N_KEYS, 2 * D], BF16,
                          kind="ExternalInput")
    id01_d = nc.dram_tensor("id01", [128, 128], BF16, kind="ExternalInput")
    shf_d = nc.dram_tensor("shf", [128, 8, 128], BF16, kind="ExternalInput")
    iota8_d = nc.dram_tensor("iota8", [128, 8], U16, kind="ExternalInput")
    out_d = nc.dram_tensor("out", [BC, D], F32, kind="ExternalOutput")

    with TileContext(nc) as tc:
        with (
            tc.tile_pool(name="const", bufs=1) as cpool,
            tc.tile_pool(name="psq", bufs=2, space="PSUM") as psq,
            tc.tile_pool(name="pss", bufs=2, space="PSUM") as pss,
            tc.tile_pool(name="pshf", bufs=2, space="PSUM") as pshfp,
            tc.tile_pool(name="st2", bufs=1) as st2,
            tc.tile_pool(name="eqs", bufs=2) as eqs,
            tc.tile_pool(name="wbp", bufs=5 if use_dg else 32) as wbp,
            tc.tile_pool(name="scr", bufs=4) as scrp,
            tc.tile_pool(name="dgp", bufs=4) as dgp,
            tc.tile_pool(name="pacc", bufs=1, space="PSUM") as paccp,
            tc.tile_pool(name="accp", bufs=2) as accp,
        ):
            if use_dg:
                nc.gpsimd.load_library(library_config.mlp)
            # ---- constant loads (ordered by first use) ----
            wqh_sb = {}
            xt_sb = []
            for k in range(4):
                t2 = cpool.tile([128, BC], F32, tag=f"xt{k}")
                nc.sync.dma_start(out=t2[:], in_=xt_d[k * 128:(k + 1) * 128, :])
                xt_sb.append(t2)
            for ch in range(2):
                for k in range(4):
                    t = cpool.tile([128, 512], F32, tag=f"wq{k}_{ch}")
                    nc.sync.dma_start(
                        out=t[:],
                        in_=wq_d[k * 128:(k + 1) * 128,
                                 ch * 512:(ch + 1) * 512])
                    wqh_sb[(k, ch)] = t
            bqp_sb = cpool.tile([D_KEYS, N_HEADS], F32, tag="bqp")
            nc.sync.dma_start(out=bqp_sb[:], in_=bqp_d[:, :])
            kbd_sb = cpool.tile([D_KEYS, N_HEADS, 2, N_KEYS], F32, tag="kbd")
            nc.sync.dma_start(out=kbd_sb[:], in_=kbd_d[:, :, :, :])
            iota8 = cpool.tile([128, 8], U16, tag="iota8")
            nc.sync.dma_start(out=iota8[:], in_=iota8_d[:, :])
            shf_sb = cpool.tile([128, 8, 128], BF16, tag="shf")
            nc.sync.dma_start(out=shf_sb[:], in_=shf_d[:, :, :])
            xtok_bf = []
            for tb in range(TB):
                tb16 = cpool.tile([128, D], BF16, tag=f"xtokb{tb}")
                nc.sync.dma_start(out=tb16[:], in_=xtokb_d[tb * 128:(tb + 1) * 128, :])
                xtok_bf.append(tb16)
            id01_sb = cpool.tile([128, 128], BF16, tag="id01")
            nc.sync.dma_start(out=id01_sb[:], in_=id01_d[:, :])

            qts = [cpool.tile([D_KEYS, N_HEADS, 128], F32, tag=f"qt{tb}",
                              name=f"qt{tb}") for tb in range(TB)]
            widx = {}    # (tb, H) -> wrapped-index tile [128, 4, NCOL] U16
            idx32 = {}   # (tb, op) -> [128, 1] I32 offsets
            ws = {tb: st2.tile([128, 64], F32, tag=f"w8{tb}", name=f"w8{tb}")
                  for tb in range(TB)}
            pages = {}

            stage1 = {}

            def route_half(tb, H):
                # per head: q projection, scores, stage-1 top8 (exact),
                # interleaved so head m's top-8 starts while head m+1 is
                # still on the PE
                tsl = slice(tb * 128, (tb + 1) * 128)
                qt = qts[tb]
                s1t = st2.tile([128, 32], F32, tag=f"s1t{tb}{H}",
                               name=f"s1t{tb}{H}")
                s2t = st2.tile([128, 32], F32, tag=f"s2t{tb}{H}",
                               name=f"s2t{tb}{H}")
                i1 = st2.tile([128, 32], U16, tag=f"i1{tb}{H}",
                              name=f"i1{tb}{H}")
                i2 = st2.tile([128, 32], U16, tag=f"i2{tb}{H}",
                              name=f"i2{tb}{H}")
                for mm in range(4):
                    m = 4 * H + mm
                    ps = psq.tile([128, 128], F32, tag="psq")
                    for k in range(4):
                        nc.tensor.matmul(
                            out=ps[:],
                            lhsT=wqh_sb[(k, H)][:, mm * 128:(mm + 1) * 128],
                            rhs=xt_sb[k][:, tsl],
                            start=(k == 0),
                            stop=(k == 3),
                        )
                    nc.vector.tensor_scalar(
                        out=qt[:, m, :], in0=ps[:],
                        scalar1=bqp_sb[:, m:m + 1], scalar2=None, op0=OP.add,
                    )
                    ps2 = pss.tile([128, 2, N_KEYS], F32, tag="pss")
                    nc.tensor.matmul(
                        out=ps2[:, :, :].rearrange("p a b -> p (a b)"),
                        lhsT=qt[:, m, :],
                        rhs=kbd_sb[:, m, :, :].rearrange("p a b -> p (a b)"),
                        start=True, stop=True,
                    )
                    for half, (st_, ix) in enumerate(((s1t, i1), (s2t, i2))):
                        nc.vector.max(out=st_[:, mm * 8:(mm + 1) * 8],
                                      in_=ps2[:, half, :])
                        nc.vector.max_index(
                            out=ix[:, mm * 8:(mm + 1) * 8],
                            in_max=st_[:, mm * 8:(mm + 1) * 8],
                            in_values=ps2[:, half, :],
                        )
                stage1[(tb, H)] = (s1t, s2t, i1, i2)

            def resolve_half(tb, H):
                s1t, s2t, i1, i2 = stage1[(tb, H)]
                # ---- stage-2: 8x8 combo scores, top8 of 64 per head ----
                cs = st2.tile([128, 256], F32, tag=f"cs{tb}{H}",
                              name=f"cs{tb}{H}")
                for mm in range(4):
                    nc.vector.tensor_tensor(
                        out=cs[:, mm * 64:(mm + 1) * 64].rearrange(
                            "p (a b) -> p a b", a=8),
                        in0=s1t[:, mm * 8:(mm + 1) * 8].unsqueeze(2)
                            .to_broadcast([128, 8, 8]),
                        in1=s2t[:, mm * 8:(mm + 1) * 8].unsqueeze(1)
                            .to_broadcast([128, 8, 8]),
                        op=OP.add,
                    )
                v8 = st2.tile([128, 32], F32, tag=f"v8{tb}{H}",
                              name=f"v8{tb}{H}")
                n8 = st2.tile([128, 32], U16, tag=f"n8{tb}{H}",
                              name=f"n8{tb}{H}")
                for mm in range(4):
                    nc.vector.max(out=v8[:, mm * 8:(mm + 1) * 8],
                                  in_=cs[:, mm * 64:(mm + 1) * 64])
                    nc.vector.max_index(
                        out=n8[:, mm * 8:(mm + 1) * 8],
                        in_max=v8[:, mm * 8:(mm + 1) * 8],
                        in_values=cs[:, mm * 64:(mm + 1) * 64])
                k1 = st2.tile([128, 32], U16, tag=f"k1{tb}{H}",
                              name=f"k1{tb}{H}")
                nc.vector.tensor_scalar(
                    out=k1[:], in0=n8[:], scalar1=3, scalar2=None,
                    op0=OP.logical_shift_right)
                k2 = st2.tile([128, 32], U16, tag=f"k2{tb}{H}",
                              name=f"k2{tb}{H}")
                nc.vector.tensor_scalar(
                    out=k2[:], in0=n8[:], scalar1=7, scalar2=None,
                    op0=OP.bitwise_and)

                # resolve winners' sub-key ids: isel[p,m,j] = i[p,m,k1[p,m,j]]
                sels = []
                for kk, ix in ((k1, i1), (k2, i2)):
                    eq = eqs.tile([128, 256], U16, tag="eq")
                    nc.vector.tensor_tensor(
                        out=eq[:, :].rearrange("p (m j k) -> p m j k", m=4, j=8),
                        in0=kk[:, :].rearrange("p (m j) -> p m j", m=4)
                            .unsqueeze(3).to_broadcast([128, 4, 8, 8]),
                        in1=iota8[:, :].unsqueeze(1).unsqueeze(1)
                            .to_broadcast([128, 4, 8, 8]),
                        op=OP.is_equal)
                    prod = eqs.tile([128, 256], U16, tag="prod")
                    nc.vector.tensor_tensor(
                        out=prod[:, :].rearrange("p (m j k) -> p m j k", m=4, j=8),
                        in0=eq[:, :].rearrange("p (m j k) -> p m j k", m=4, j=8),
                        in1=ix[:, :].rearrange("p (m k) -> p m k", m=4)
                            .unsqueeze(2).to_broadcast([128, 4, 8, 8]),
                        op=OP.mult)
                    sel = st2.tile([128, 32], U16, tag=f"sel{len(sels)}{tb}{H}",
                                   name=f"sel{len(sels)}{tb}{H}")
                    with nc.allow_low_precision(
                            reason="one-hot uint16 sum, values <= 255"):
                        nc.vector.reduce_sum(
                            out=sel[:],
                            in_=prod[:, :].rearrange("p (mj k) -> p mj k", k=8),
                            axis=X)
                    sels.append(sel)

                if use_dg:
                    # ---- wrap sub-key ids into dma_gather idx layout ----
                    selb = st2.tile([128, 64], BF16, tag=f"selb{tb}{H}",
                                    name=f"selb{tb}{H}")
                    nc.vector.tensor_copy(out=selb[:, 0:32], in_=sels[0][:])
                    nc.vector.tensor_copy(out=selb[:, 32:64], in_=sels[1][:])
                    w12 = st2.tile([128, 2, 4, NCOL], U16, tag=f"w12{tb}{H}",
                                   name=f"w12{tb}{H}")
                    for hi in range(8):
                        pf = pshfp.tile([128, 64], F32, tag="pshf")
                        nc.tensor.matmul(
                            out=pf[:], lhsT=shf_sb[:, hi, :], rhs=selb[:],
                            start=True, stop=True)
                        # W[q, s, oo, j*8+hi] =
                        #   sel_s[16*hi + q%16, slot 8(4H+oo)+j]
                        nc.vector.tensor_copy(
                            out=w12[:, :, :, hi:hi + 49:8],
                            in_=pf[:, :].rearrange(
                                "p (s o j) -> p s o j", s=2, o=4)[:, :, :, 0:7])
                    wx = st2.tile([128, 4, NCOL], U16, tag=f"wx{tb}{H}",
                                  name=f"wx{tb}{H}")
                    with nc.allow_low_precision(
                            reason="u16 expert-id packing, wraps by design"):
                        nc.vector.scalar_tensor_tensor(
                            out=wx[:], in0=w12[:, 0], scalar=256,
                            in1=w12[:, 1], op0=OP.mult, op1=OP.add)
                        nc.vector.tensor_scalar(
                            out=wx[:], in0=wx[:], scalar1=32768, scalar2=None,
                            op0=OP.bitwise_xor)
                        # pad column: idx 0 (>=0 as int16) at position 896
                        # defeats the trailing-negative trim
                        nc.vector.tensor_scalar(
                            out=wx[:, :, 56:57], in0=wx[:, :, 56:57],
                            scalar1=0, scalar2=None, op0=OP.mult)
                    widx[(tb, H)] = wx
                    # repair indices: the 8th slot of each head, gathered
                    # via plain single-offset indirect DMA (exact)
                    idxr = st2.tile([128, 4], U16, tag=f"idxr{tb}{H}",
                                    name=f"idxr{tb}{H}")
                    with nc.allow_low_precision(
                            reason="u16 expert-id packing, < 65536"):
                        nc.vector.tensor_scalar(
                            out=idxr[:], in0=sels[0][:, 7:32:8], scalar1=256,
                            scalar2=None, op0=OP.mult)
                        nc.vector.tensor_tensor(
                            out=idxr[:], in0=idxr[:], in1=sels[1][:, 7:32:8],
                            op=OP.add)
                    for oo in range(4):
                        ixg = st2.tile([128, 1], I32, tag=f"ixr{tb}{H}{oo}",
                                       name=f"ixr{tb}{H}{oo}")
                        nc.vector.tensor_copy(out=ixg[:],
                                              in_=idxr[:, oo:oo + 1])
                        idx32[(tb, 4 * H + oo)] = ixg
                else:
                    idx16 = st2.tile([128, 32], U16, tag=f"idx16{tb}{H}",
                                     name=f"idx16{tb}{H}")
                    nc.vector.tensor_scalar(
                        out=idx16[:], in0=sels[0][:], scalar1=256, scalar2=None,
                        op0=OP.mult)
                    nc.vector.tensor_tensor(
                        out=idx16[:], in0=idx16[:], in1=sels[1][:], op=OP.add)
                    for o in range(32):
                        s = 32 * H + o
                        ixg = st2.tile([128, 1], I32, tag=f"ixg{tb}_{s}",
                                       name=f"ixg{tb}_{s}")
                        nc.vector.tensor_copy(out=ixg[:],
                                              in_=idx16[:, o:o + 1])
                        idx32[(tb, s)] = ixg

                # ---- softmax over each head's top-8 ----
                rmax = st2.tile([128, 4], F32, tag=f"rmax{tb}{H}",
                                name=f"rmax{tb}{H}")
                nc.vector.reduce_max(
                    out=rmax[:], in_=v8[:, :].rearrange("p (m k) -> p m k", m=4),
                    axis=X)
                ex = st2.tile([128, 32], F32, tag=f"ex{tb}{H}",
                              name=f"ex{tb}{H}")
                nc.vector.tensor_tensor(
                    out=ex[:, :].rearrange("p (m k) -> p m k", m=4),
                    in0=v8[:, :].rearrange("p (m k) -> p m k", m=4),
                    in1=rmax[:, :].unsqueeze(2).to_broadcast([128, 4, 8]),
                    op=OP.subtract)
                nc.scalar.activation(
                    out=ex[:], in_=ex[:], func=mybir.ActivationFunctionType.Exp)
                rsum = st2.tile([128, 4], F32, tag=f"rsum{tb}{H}",
                                name=f"rsum{tb}{H}")
                nc.vector.reduce_sum(
                    out=rsum[:], in_=ex[:, :].rearrange("p (m k) -> p m k", m=4),
                    axis=X)
                rinv = st2.tile([128, 4], F32, tag=f"rinv{tb}{H}",
                                name=f"rinv{tb}{H}")
                nc.vector.reciprocal(out=rinv[:], in_=rsum[:])
                nc.vector.tensor_tensor(
                    out=ws[tb][:, 32 * H:32 * H + 32].rearrange(
                        "p (m k) -> p m k", m=4),
                    in0=ex[:, :].rearrange("p (m k) -> p m k", m=4),
                    in1=rinv[:, :].unsqueeze(2).to_broadcast([128, 4, 8]),
                    op=OP.mult)

            def issue_gathers(tb, H):
                if use_dg:
                    for oo in range(4):
                        o = 4 * H + oo
                        page = wbp.tile([128, 8, 1024], BF16, tag="wbpage",
                                        name=f"pg{tb}_{o}")
                        pages[(tb, o)] = page
                        nc.gpsimd.dma_gather(
                            out_ap=page[:],
                            in_ap=wb_d[32768:, :],
                            idxs_ap=widx[(tb, H)][:, oo, :].bitcast(I16),
                            num_idxs=897,
                            num_idxs_reg=897,
                            elem_size=1024,
                        )
                        pic = wbp.tile([128, 1024], BF16, tag="wbic",
                                       name=f"pic{tb}_{o}", bufs=8)
                        pages[(tb, o, "ic")] = pic
                        nc.gpsimd.indirect_dma_start(
                            out=pic[:], out_offset=None,
                            in_=wb_d[:, :],
                            in_offset=IndirectOffsetOnAxis(
                                ap=idx32[(tb, o)][:], axis=0),
                        )
                else:
                    for o in range(32):
                        s = 32 * H + o
                        page = wbp.tile([128, 1024], BF16, tag="wbpage",
                                        name=f"pg{tb}_{s}")
                        pages[(tb, s)] = page
                        nc.gpsimd.indirect_dma_start(
                            out=page[:], out_offset=None,
                            in_=wb_d[:, :],
                            in_offset=IndirectOffsetOnAxis(
                                ap=idx32[(tb, s)][:], axis=0),
                        )

            def get_page_slice(tb, grp, sidx, lo, hi):
                s = grp * 8 + sidx
                if use_dg:
                    if sidx == 7:
                        return pages[(tb, grp, "ic")][:, lo:hi]
                    return pages[(tb, grp)][:, sidx, lo:hi]
                return pages[(tb, s)][:, lo:hi]

            inner_t = {}
            va_t = {}
            pacc_t = {}

            def compute_half(tb, H):
                if H == 0:
                    inner_t[tb] = st2.tile([128, 64], F32, tag=f"inner{tb}",
                                           name=f"inner{tb}")
                    va_t[tb] = st2.tile([128, 64], F32, tag=f"va{tb}",
                                        name=f"va{tb}")
                    pacc_t[tb] = paccp.tile([128, D], F32, tag=f"pacc{tb}",
                                            name=f"pacc{tb}")
                inner, va, pacc = inner_t[tb], va_t[tb], pacc_t[tb]
                for grp in range(4 * H, 4 * H + 4):
                    gs = slice(grp * 8, (grp + 1) * 8)
                    for sidx in range(8):
                        col = grp * 8 + sidx
                        wdrow = get_page_slice(tb, grp, sidx, 0, D)
                        scr = scrp.tile([128, D], BF16, tag="scr")
                        if sidx < stt_split(tb, grp):
                            nc.vector.scalar_tensor_tensor(
                                out=scr[:], in0=wdrow, scalar=1.0,
                                in1=xtok_bf[tb][:], op0=OP.mult, op1=OP.mult,
                                accum_out=inner[:, col:col + 1])
                        else:
                            nc.vector.tensor_tensor(
                                out=scr[:], in0=wdrow,
                                in1=xtok_bf[tb][:], op=OP.mult)
                            scr2 = scrp.tile([128, D], BF16, tag="scr2")
                            nc.scalar.activation(
                                out=scr2[:], in_=scr[:],
                                func=mybir.ActivationFunctionType.Copy,
                                accum_out=inner[:, col:col + 1])
                    rl8 = st2.tile([128, 8], F32, tag=f"rl{tb}", name=f"rl{tb}")
                    nc.scalar.activation(
                        out=rl8[:], in_=inner[:, gs],
                        func=mybir.ActivationFunctionType.Relu)
                    nc.vector.tensor_tensor(
                        out=va[:, gs], in0=rl8[:], in1=ws[tb][:, gs],
                        op=OP.mult)
                    # all 8 diag matrices of the group in one DVE op
                    dgrp = dgp.tile([128, 8, 128], BF16, tag="dgrp")
                    nc.vector.tensor_tensor(
                        out=dgrp[:],
                        in0=va[:, gs].unsqueeze(2).to_broadcast([128, 8, 128]),
                        in1=id01_sb[:].unsqueeze(1).to_broadcast([128, 8, 128]),
                        op=OP.mult)
                    for sidx in range(8):
                        col = grp * 8 + sidx
                        nc.tensor.matmul(
                            out=pacc[:], lhsT=dgrp[:, sidx, :],
                            rhs=get_page_slice(tb, grp, sidx, D, 2 * D),
                            start=(col == 0), stop=(col == 63))
                if H == 1:
                    acc_sb = accp.tile([128, D], F32, tag=f"acc{tb}",
                                       name=f"acc{tb}")
                    nc.vector.tensor_copy(out=acc_sb[:], in_=pacc[:])
                    nc.sync.dma_start(
                        out=out_d[tb * 128:(tb + 1) * 128, :], in_=acc_sb[:])

            route_half(0, 0)
            resolve_half(0, 0)
            issue_gathers(0, 0)
            route_half(0, 1)
            resolve_half(0, 1)
            issue_gathers(0, 1)
            route_half(1, 0)
            resolve_half(1, 0)
            compute_half(0, 0)
            issue_gathers(1, 0)
            route_half(1, 1)
            resolve_half(1, 1)
            compute_half(0, 1)
            issue_gathers(1, 1)
            compute_half(1, 0)
            compute_half(1, 1)

    nc.compile()
    return nc


_NC_CACHE = None
VARIANT = {}


def _get_nc():
    global _NC_CACHE
    if _NC_CACHE is None:
        _NC_CACHE = build_nc(**VARIANT)
    return _NC_CACHE


def _prep_in_maps(inputs):
    q = np.ascontiguousarray(np.asarray(inputs["queries"], dtype=np.float32))
    Wq = np.ascontiguousarray(np.asarray(inputs["Wq"], dtype=np.float32))
    bq = np.asarray(inputs["bq"], dtype=np.float32)
    keys = np.asarray(inputs["keys"], dtype=np.float32)
    wd = np.asarray(inputs["w_down"], dtype=np.float32)
    wu = np.asarray(inputs["w_up"], dtype=np.float32)
    import ml_dtypes
    wb = np.ascontiguousarray(
        np.concatenate([wd, wu], axis=1).astype(ml_dtypes.bfloat16))
    id01 = np.eye(128, dtype=np.float32).astype(ml_dtypes.bfloat16)

    x = q.reshape(B, D)
    # bqp[p, m] = bq[m*128 + p]
    bqp = np.ascontiguousarray(bq.reshape(N_HEADS, D_KEYS).T)
    # block-diagonal keys: kbd[p, m, 0, n] = keys[m, 0, n, p] for p < 64,
    # kbd[p, m, 1, n] = keys[m, 1, n, p-64] for p >= 64, zero elsewhere.
    kbd = np.zeros((D_KEYS, N_HEADS, 2, N_KEYS), np.float32)
    kbd[:HALF, :, 0, :] = keys[:, 0].transpose(2, 0, 1)
    kbd[HALF:, :, 1, :] = keys[:, 1].transpose(2, 0, 1)
    kbd = np.ascontiguousarray(kbd)
    # shuffle/replicate matrices: shf[p, hi, f] = 1 iff p == 16*hi + f%16
    shf = np.zeros((128, 8, 128), np.float32)
    for hi in range(8):
        for f in range(128):
            shf[16 * hi + f % 16, hi, f] = 1.0
    shf = np.ascontiguousarray(shf.astype(ml_dtypes.bfloat16))
    iota8 = np.broadcast_to(np.arange(8, dtype=np.uint16), (128, 8))
    iota8 = np.ascontiguousarray(iota8)

    in_maps = []
    for c in range(N_CORES):
        xc = x[c * BC:(c + 1) * BC]
        in_maps.append({
            "xtokb": np.ascontiguousarray(xc.astype(ml_dtypes.bfloat16)),
            "xt": np.ascontiguousarray(xc.T),
            "wq": Wq,
            "bqp": bqp,
            "kbd": kbd,
            "wb": wb,
            "id01": id01,
            "shf": shf,
            "iota8": iota8,
        })
    return in_maps


def run(inputs, trace=False):
    """Run on 8 NeuronCores; returns (out [2,1024,512], BassKernelResults)."""
    nc = _get_nc()
    in_maps = _prep_in_maps(inputs)
    res = run_bass_kernel_spmd(
        nc, in_maps, core_ids=list(range(N_CORES)), trace=trace)
    out = np.concatenate(
        [res.results[c]["out"] for c in range(N_CORES)], axis=0)
    return out.reshape(2, 1024, D), res


def kernel(**inputs) -> np.ndarray:
    out, _ = run(inputs, trace=False)
    return out
